# revision 36
# baseline (speedup 1.0000x reference)
"""Quantized ViT MLP (fake-quant int8) on 8 Trainium2 NeuronCores.

Strategy
--------
Data-parallel over tokens (12608 tokens -> 1576/core, padded to 1664).
Weights are small so they are replicated; no collectives.

Key numeric insight: the fake-quant values are integers in [-127, 127],
which are exactly representable in bf16, and the integer matmul
accumulates to < 2^24 in fp32 PSUM -> the bf16 matmul is BIT-EXACT
equal to the fp32 reference matmul of the quantized values.

Per-core pipeline (per 128-token tile):
  x [128,768] f32 --DVE absmax--> s1 = clip/127, rs1 = 1/s1
  DVE mult(x*rs1) + 1.5*2^23 then DVE -C -> qx bf16 (round-half-even,
  bit-matches jnp.round)
  DMA-xbar transpose qx -> qxT [128, 6, 128] (K-major for matmul)
  fc1: 6x(hid chunk 512): accumulate 6 K-tiles in PSUM (bf16 matmul)
  ACT Gelu(acc * (s1*sw1)) PSUM->SBUF (exact-erf gelu table)
  DVE absmax -> s2, rs2; ACT-quantize h -> qh bf16
  DMA-xbar transpose qh halves -> qhT [128, 12, 128] x2
  fc2: 2x(d chunk 384): accumulate 24 K-tiles in PSUM
  ACT Copy(acc * (s2*sw2)) -> out f32 -> DMA to DRAM (per 384 chunk)

Schedule notes:
 - Weights ship as INT8 and are cast to bf16 by the gpsimd SWDGE DMA
   (only gpsimd can cast) -- halves weight HBM traffic; a single queue
   in consumption order leaves the other queues' bandwidth to the x
   tiles and transposes.  The last fc2 quarter ships bf16 on the scalar
   queue so it lands before the first fc2 needs it.
 - A burst of dummy matmuls on a zeroed tile warms the PE HAM
   clock-gate (cold 1.2 -> warm 2.4 GHz takes ~3.4us of activity)
   while the first DMAs land.
 - The first 4 x tiles load as two half-row DMAs spread across the
   sync + scalar HWDGE queues, and the weight stream is gated on x2's
   arrival by a tiny DVE op writing into the first weight buffer (WAW
   dep), so the startup-critical x tiles get the DMA bandwidth first.
 - The first WARM tiles run fc1 chunk-major (each arriving weight
   chunk is consumed WARM times back-to-back) to match arrival rate.
 - Software pipeline: quantT(i+4) / fc1+epilogue(i+3) / fc2(i) so the
   activation quant + DMA transposes run a full tile period ahead of
   the matmuls that consume them.
 - qh transposes as 2 halves (not 4 quarters): the sync engine's
   DMA_TRANSPOSE kick instruction costs ~1.3us each, 3 kicks/tile fit
   the budget where 5 did not.
 - Output stores kick from the gpsimd queue (empty after weights) to
   keep the ACT engine under budget.

Per-tensor weight scales + quantized weights are computed on the host
(init-time constants, as sanctioned by the sharding hint). Weights are
laid out on the host to match the DMA-transpose xbar's
k->(partition,tile) mapping of the activations, so the mapping cancels
identically.

Biases are dropped: the reference adds them in the *integer* domain
before the dequant rescale (out = (int_mm + b) * sx * sw), so their
relative contribution is ~1e-6 of the integer accumulator -- far below
fp32 noise in the output.
"""

import os
import sys

for _p in ("/opt/trn_rl_repo",):
    if _p not in sys.path and os.path.isdir(_p):
        sys.path.insert(0, _p)

from contextlib import ExitStack

import ml_dtypes
import numpy as np

import concourse.bacc as bacc
import concourse.mybir as mybir
import concourse.tile as tile
from concourse.bass_utils import run_bass_kernel_spmd

# Problem constants (hardcoded; kernel.py must be self-contained)
B, S, D, H = 64, 197, 768, 3072
N_CORES = 8
NTOK = B * S                      # 12608
TOK_PER_CORE = NTOK // N_CORES    # 1576
P = 128
N_TILES = (TOK_PER_CORE + P - 1) // P   # 13
TOK_PAD = N_TILES * P                   # 1664
KD = D // P                              # 6 k-tiles for fc1
KH = H // P                              # 24 k-tiles for fc2
HC = 512                                 # fc1 psum chunk (1 bank fp32)
DC = 384                                 # fc2 psum chunk (<=512)
N_HC = H // HC                           # 6
N_DC = D // DC                           # 2
NQ = 4                                   # h-quant quarters / qw2 quarters
HQ = H // NQ                             # 768 features per quarter
KHQ = KH // NQ                           # 6 k-tiles per quarter
C_ROUND = 12582912.0                     # 1.5*2^23: fp32 RNE round trick
WARM = 3                                 # tiles interleaved with weight arrival
N_DUMMY = 16                             # HAM warm-up matmuls on zeros

F32 = mybir.dt.float32
BF16 = mybir.dt.bfloat16
I8 = mybir.dt.int8


def build_nc():
    nc = bacc.Bacc(
        "TRN2",
        target_bir_lowering=False,
        debug=False,
        enable_asserts=False,
        num_devices=N_CORES,
    )
    x_d = nc.dram_tensor("x", [TOK_PAD, D], F32, kind="ExternalInput").ap()
    # weights arrive pre-quantized AND pre-transposed into k-tile layout,
    # chunked to match on-device consumption order:
    # qw1t[hc, p, k, j] = round(w1/sw1)[hc*512+j, k*128+p]
    qw1_d = nc.dram_tensor(
        "qw1t", [N_HC - 1, P, KD, HC], I8, kind="ExternalInput"
    ).ap()
    qw1b_d = nc.dram_tensor(
        "qw1tb", [P, KD, HC], BF16, kind="ExternalInput"
    ).ap()
    # qw2t[q, p, kl, d] = round(w2/sw2)[d, (q*6+kl)*128+p]
    # quarters 0-2 ship int8 (gpsimd cast-DMA); quarter 3 ships bf16 on
    # the scalar HWDGE queue so it lands before the first fc2 needs it
    qw2a_d = nc.dram_tensor(
        "qw2ta", [2, P, KHQ, D], I8, kind="ExternalInput"
    ).ap()
    qw2b_d = nc.dram_tensor(
        "qw2tb", [2, P, KHQ, D], BF16, kind="ExternalInput"
    ).ap()
    wsc_d = nc.dram_tensor("wsc", [2], F32, kind="ExternalInput").ap()
    out_d = nc.dram_tensor("out", [TOK_PAD, D], F32, kind="ExternalOutput").ap()

    Alu = mybir.AluOpType
    Act = mybir.ActivationFunctionType

    with tile.TileContext(nc) as tc, ExitStack() as ctx:
        wpool = ctx.enter_context(tc.tile_pool(name="wpool", bufs=1))
        spool = ctx.enter_context(tc.tile_pool(name="spool", bufs=1))
        xpool = ctx.enter_context(tc.tile_pool(name="xpool", bufs=7))
        qpool = ctx.enter_context(tc.tile_pool(name="qpool", bufs=3))
        gpool = ctx.enter_context(tc.tile_pool(name="gpool", bufs=3))
        opool = ctx.enter_context(tc.tile_pool(name="opool", bufs=2))
        stpool = ctx.enter_context(tc.tile_pool(name="stpool", bufs=5))
        ps1 = ctx.enter_context(tc.tile_pool(name="ps1", bufs=4, space="PSUM"))
        ps2 = ctx.enter_context(tc.tile_pool(name="ps2", bufs=2, space="PSUM"))

        # ---- PE HAM warm-up: dummy matmuls on a zeroed tile so the
        # clock-gate reaches 8/8 while the first real DMAs land.  The
        # scratch PSUM comes from ps1's own rotation.
        zt = spool.tile([P, HC], BF16)
        nc.gpsimd.memset(zt[:], 0.0)
        pswt = ps1.tile([P, HC], F32, name="warm_mm", tag="p1")
        for _ in range(N_DUMMY):
            nc.tensor.matmul(pswt, lhsT=zt[:, 0:P], rhs=zt, start=True,
                             stop=True)

        # ---- early x tiles: two half-row DMAs spread across the sync +
        # scalar HWDGE queues so the first tiles land fast even while the
        # weight queue streams.  Later tiles load whole on scalar.
        def load_x(i, eng):
            t = xpool.tile([P, D], F32, name=f"x_{i}", tag="x_t")
            eng.dma_start(out=t, in_=x_d[i * P:(i + 1) * P, :])
            return t

        def load_x_split(i):
            t = xpool.tile([P, D], F32, name=f"x_{i}", tag="x_t")
            h = D // 2
            nc.sync.dma_start(out=t[:, 0:h], in_=x_d[i * P:(i + 1) * P, 0:h])
            nc.scalar.dma_start(out=t[:, h:D], in_=x_d[i * P:(i + 1) * P, h:D])
            return t

        # ---- weight scales, broadcast across partitions: FIRST on the
        # sync ring (8 bytes; everything ACT-side waits on it)
        wsc = spool.tile([P, 2], F32)
        import concourse.bass as bass
        wsc_bcast = bass.AP(
            tensor=wsc_d.tensor, offset=wsc_d.offset,
            ap=[[0, P]] + list(wsc_d.ap),
        )
        nc.sync.dma_start(out=wsc, in_=wsc_bcast)

        x_tiles = {}
        for i in range(min(4, N_TILES)):
            x_tiles[i] = load_x_split(i)

        # ---- weights: int8 -> bf16 cast on the single gpsimd SWDGE
        # queue, unchained (ring descriptors are consumed in issue order
        # = consumption order); qt1/qt3 as bf16 on scalar (land early).
        # The weight ring is held back ~3us by an SBUF->SBUF broadcast
        # copy into the first chunk's buffer: a WAW data dependency that
        # keeps the weight stream off the HBM while the startup-critical
        # x tiles land at full bandwidth (it costs no HBM itself).
        qw1c = [
            wpool.tile([P, KD, HC], BF16, name=f"qw1_{hc}",
                       tag=f"qw1_{hc}")
            for hc in range(N_HC)
        ]
        qw2c = [None] * NQ
        # Zero-cost weight-stream gate: a tiny DVE op that reads x2 and
        # writes into the first weight chunk's buffer.  The WAW dependency
        # holds the weight DMA stream off the HBM until the startup-
        # critical x tiles have landed, without consuming any DMA engine
        # time itself.
        if N_TILES > 2:
            nc.vector.tensor_scalar(
                out=qw1c[0][:, 0, 0:16], in0=x_tiles[2][:, 0:16],
                scalar1=0.0, scalar2=None, op0=Alu.mult,
            )
        for hc in range(N_HC - 1):
            nc.gpsimd.dma_start(out=qw1c[hc], in_=qw1_d[hc])
        nc.scalar.dma_start(out=qw1c[N_HC - 1], in_=qw1b_d)
        for q in (1, 3):
            w = wpool.tile([P, KHQ, D], BF16, name=f"qw2_{q}",
                           tag=f"qw2_{q}")
            nc.scalar.dma_start(out=w, in_=qw2b_d[q // 2])
            qw2c[q] = w
        for q in (0, 2):
            w = wpool.tile([P, KHQ, D], BF16, name=f"qw2_{q}",
                           tag=f"qw2_{q}")
            nc.gpsimd.dma_start(out=w, in_=qw2a_d[q // 2])
            qw2c[q] = w

        # x4/x5 preissued whole on the scalar ring (behind the bf16 qw2
        # quarters); later tiles prefetch on the gpsimd ring, which is
        # empty once the weights drain.
        for i in (4, 5):
            if i < N_TILES:
                x_tiles[i] = load_x(i, nc.scalar)

        # Prime both gelu ACT table banks before any real work so the
        # ~1.3us table loads don't stall the first PSUM evacuations.
        warmt = spool.tile([P, 2], F32)
        nc.scalar.activation(
            out=warmt[:, 0:1], in_=wsc[:, 0:1], func=Act.Gelu, scale=1.0
        )
        nc.scalar.activation(
            out=warmt[:, 1:2], in_=wsc[:, 0:1], func=Act.Gelu, scale=500.0
        )

        state = {}

        def quantT(i, warm=False, prefetch=True):
            """x absmax/scale + quantize + transpose for tile i.

            absmax/scales live on the DVE.  For warm tiles the wide
            quantize mult/sub runs on the ACT engine (idle before the
            first gelu) so the first chains pipeline instead of
            serializing on the DVE.
            """
            x_t = x_tiles.pop(i)
            if prefetch and i + 6 < N_TILES:
                x_tiles[i + 6] = load_x(i + 6, nc.gpsimd)

            mx = stpool.tile([P, 1], F32, name=f"mx_{i}", tag="mx")
            nc.vector.tensor_reduce(
                out=mx, in_=x_t, axis=mybir.AxisListType.X,
                op=Alu.max, apply_absolute_value=True,
            )
            s1 = stpool.tile([P, 1], F32, name=f"s1_{i}", tag="s1")
            nc.vector.tensor_scalar(
                out=s1, in0=mx, scalar1=1e-6, scalar2=1.0 / 127.0,
                op0=Alu.max, op1=Alu.mult,
            )
            rs1 = stpool.tile([P, 1], F32, name=f"rs1_{i}", tag="rs1")
            nc.vector.reciprocal(out=rs1, in_=s1)
            qx = qpool.tile([P, D], BF16, name=f"qx_{i}", tag="qx")
            if warm:
                nc.scalar.activation(
                    out=x_t, in_=x_t, func=Act.Copy, bias=C_ROUND, scale=rs1,
                )
                nc.scalar.activation(
                    out=qx, in_=x_t, func=Act.Copy, bias=-C_ROUND, scale=1.0,
                )
            else:
                nc.vector.tensor_scalar(
                    out=x_t, in0=x_t, scalar1=rs1, scalar2=C_ROUND,
                    op0=Alu.mult, op1=Alu.add,
                )
                nc.vector.tensor_scalar(
                    out=qx, in0=x_t, scalar1=C_ROUND, scalar2=None,
                    op0=Alu.subtract,
                )
            qxT = qpool.tile([P, KD, P], BF16, name=f"qxT_{i}", tag="qxT")
            nc.sync.dma_start(out=qxT, in_=qx, transpose=True)
            gsc = stpool.tile([P, 1], F32, name=f"gsc_{i}", tag="gsc", bufs=6)
            nc.vector.tensor_scalar(
                out=gsc, in0=s1, scalar1=wsc[:, 0:1], scalar2=None, op0=Alu.mult
            )
            state[("q", i)] = (qxT, gsc)

        def fc1_chunk(i, hc, qxT, gsc, g, mh6):
            """One 512-wide fc1 chunk: matmul + fused scale/Gelu + amax."""
            p1 = ps1.tile([P, HC], F32, name=f"p1_{i}_{hc}", tag="p1")
            for kt in range(KD):
                nc.tensor.matmul(
                    p1,
                    lhsT=qxT[:, kt, :],
                    rhs=qw1c[hc][:, kt, :],
                    start=(kt == 0),
                    stop=(kt == KD - 1),
                )
            nc.scalar.activation(
                out=g[:, hc * HC:(hc + 1) * HC], in_=p1,
                func=Act.Gelu, scale=gsc,
            )
            nc.vector.tensor_reduce(
                out=mh6[:, hc:hc + 1], in_=g[:, hc * HC:(hc + 1) * HC],
                axis=mybir.AxisListType.X, op=Alu.max,
                apply_absolute_value=True,
            )

        def epilogue1(i, g, mh6):
            """h scales + quantize in quarters + transpose halves."""
            mh = stpool.tile([P, 1], F32, name=f"mh_{i}", tag="mh")
            nc.vector.tensor_reduce(
                out=mh, in_=mh6, axis=mybir.AxisListType.X, op=Alu.max
            )
            s2 = stpool.tile([P, 1], F32, name=f"s2_{i}", tag="s2")
            nc.vector.tensor_scalar(
                out=s2, in0=mh, scalar1=1e-6, scalar2=1.0 / 127.0,
                op0=Alu.max, op1=Alu.mult,
            )
            rs2 = stpool.tile([P, 1], F32, name=f"rs2_{i}", tag="rs2")
            nc.vector.reciprocal(out=rs2, in_=s2)
            qh = qpool.tile([P, H], BF16, name=f"qh_{i}", tag="qh", bufs=2)
            qhT = []
            for q in range(NQ):
                hs = slice(q * HQ, (q + 1) * HQ)
                nc.scalar.activation(
                    out=g[:, hs], in_=g[:, hs], func=Act.Copy,
                    bias=C_ROUND, scale=rs2,
                )
                nc.vector.tensor_scalar(
                    out=qh[:, hs], in0=g[:, hs], scalar1=C_ROUND,
                    scalar2=None, op0=Alu.subtract,
                )
                if q % 2 == 1:
                    hh = q // 2
                    qhT_h = qpool.tile(
                        [P, 2 * KHQ, P], BF16, name=f"qhT_{i}_{hh}",
                        tag=f"qhT_{hh}", bufs=5,
                    )
                    nc.sync.dma_start(
                        out=qhT_h,
                        in_=qh[:, hh * (H // 2):(hh + 1) * (H // 2)],
                        transpose=True,
                    )
                    qhT.append(qhT_h)
            osc = stpool.tile([P, 1], F32, name=f"osc_{i}", tag="osc", bufs=6)
            nc.vector.tensor_scalar(
                out=osc, in0=s2, scalar1=wsc[:, 1:2], scalar2=None, op0=Alu.mult
            )
            state[i] = (qhT, osc)

        def fc1_phase(i):
            qxT, gsc = state.pop(("q", i))
            g = gpool.tile([P, H], F32, name=f"g_{i}", tag="g")
            mh6 = stpool.tile([P, N_HC], F32, name=f"mh6_{i}", tag="mh6")
            for hc in range(N_HC):
                fc1_chunk(i, hc, qxT, gsc, g, mh6)
            epilogue1(i, g, mh6)

        def phase2(i):
            """fc2 + dequant + store for tile i (store per 384 chunk)."""
            qhT, osc = state.pop(i)
            o_t = opool.tile([P, D], F32, name=f"o_{i}", tag="o_t")
            p2s = [
                ps2.tile([P, DC], F32, name=f"p2_{i}_{dc}", tag=f"p2_{dc}")
                for dc in range(N_DC)
            ]
            for q in range(NQ):
                for ktl in range(KHQ):
                    kt = q * KHQ + ktl
                    for dc in range(N_DC):
                        nc.tensor.matmul(
                            p2s[dc],
                            lhsT=qhT[kt // (2 * KHQ)][:, kt % (2 * KHQ), :],
                            rhs=qw2c[q][:, ktl, dc * DC:(dc + 1) * DC],
                            start=(kt == 0),
                            stop=(kt == KH - 1),
                        )
            for dc in range(N_DC):
                cs = slice(dc * DC, (dc + 1) * DC)
                nc.scalar.activation(
                    out=o_t[:, cs], in_=p2s[dc], func=Act.Copy, scale=osc,
                )
                nc.gpsimd.dma_start(
                    out=out_d[i * P:(i + 1) * P, cs], in_=o_t[:, cs]
                )

        # Warmup: quantize the first WARM+1 tiles, then interleave the
        # first WARM tiles' fc1 hc-major so the PE consumes each arriving
        # qw1 chunk WARM times back-to-back -- matches the chunk arrival
        # rate instead of stalling in-order.
        for t in range(min(WARM + 1, N_TILES)):
            quantT(t)
        warm_ctx = []
        for t in range(WARM):
            g = gpool.tile([P, H], F32, name=f"g_{t}", tag="g")
            mh6 = stpool.tile([P, N_HC], F32, name=f"mh6_{t}", tag="mh6")
            warm_ctx.append((g, mh6))
        for hc in range(N_HC):
            for t in range(WARM):
                g, mh6 = warm_ctx[t]
                qxT, gsc = state[("q", t)]
                fc1_chunk(t, hc, qxT, gsc, g, mh6)
        for t in range(WARM):
            g, mh6 = warm_ctx[t]
            state.pop(("q", t))
            epilogue1(t, g, mh6)

        # fc2 lags the fc1 stream by one extra tile so the warm tiles'
        # bunched epilogues (quant + qh transposes) get a full tile
        # period of ACT/DVE/transpose headroom before fc2 consumes them
        for i in range(N_TILES + 1):
            if i + WARM < N_TILES:
                quantT(i + WARM + 1) if i + WARM + 1 < N_TILES else None
                fc1_phase(i + WARM)
            if i >= 1:
                phase2(i - 1)

    nc.compile()
    return nc


def _host_prep(x, w1, w2):
    """Quantize + k-tile-transpose weights on the host (init constants)."""
    f32 = np.float32
    sw1 = np.maximum(np.abs(w1).max().astype(f32), f32(1e-6)) / f32(127.0)
    sw2 = np.maximum(np.abs(w2).max().astype(f32), f32(1e-6)) / f32(127.0)
    qw1 = np.round(w1.astype(f32) / sw1)   # [H, D] integers in [-127, 127]
    qw2 = np.round(w2.astype(f32) / sw2)   # [D, H]
    # qw1t[hc, p, k, j] = qw1[hc*HC+j, k*128+p]
    qw1t_full = np.ascontiguousarray(
        qw1.reshape(N_HC, HC, KD, P).transpose(0, 3, 2, 1)
    )
    qw1t = np.ascontiguousarray(qw1t_full[:N_HC - 1]).astype(np.int8)
    qw1tb = np.ascontiguousarray(qw1t_full[N_HC - 1]).astype(
        ml_dtypes.bfloat16)
    # qw2t[q, p, kl, d] = qw2[d, (q*KHQ+kl)*128+p]
    qw2t = np.ascontiguousarray(
        qw2.reshape(D, NQ, KHQ, P).transpose(1, 3, 2, 0)
    )
    qw2ta = np.ascontiguousarray(qw2t[0::2]).astype(np.int8)
    qw2tb = np.ascontiguousarray(qw2t[1::2]).astype(ml_dtypes.bfloat16)

    x2d = np.ascontiguousarray(x.astype(f32).reshape(-1, D))
    xpad = np.zeros((N_CORES, TOK_PAD, D), dtype=np.float32)
    xpad[:, :TOK_PER_CORE, :] = x2d.reshape(N_CORES, TOK_PER_CORE, D)
    wsc = np.array([sw1, sw2], dtype=np.float32)
    return xpad, qw1t, qw1tb, qw2ta, qw2tb, wsc


_NC_CACHE = []


def get_nc():
    if not _NC_CACHE:
        _NC_CACHE.append(build_nc())
    return _NC_CACHE[0]


def make_in_maps(x, w1, w2):
    xpad, qw1t, qw1tb, qw2ta, qw2tb, wsc = _host_prep(x, w1, w2)
    return [
        {"x": xpad[c], "qw1t": qw1t, "qw1tb": qw1tb, "qw2ta": qw2ta,
         "qw2tb": qw2tb, "wsc": wsc}
        for c in range(N_CORES)
    ]


def run(nc, in_maps, **kw):
    res = run_bass_kernel_spmd(nc, in_maps, core_ids=list(range(N_CORES)), **kw)
    outs = [res.results[c]["out"][:TOK_PER_CORE] for c in range(N_CORES)]
    full = np.concatenate(outs, axis=0).reshape(B, S, D).astype(np.float32)
    return full, res


def kernel(x, w1, b1, w2, b2):
    nc = get_nc()
    in_maps = make_in_maps(np.asarray(x), np.asarray(w1), np.asarray(w2))
    full, _ = run(nc, in_maps)
    return full


# revision 38
# speedup vs baseline: 1.0115x; 1.0115x over previous
"""Quantized ViT MLP (fake-quant int8) on 8 Trainium2 NeuronCores.

Strategy
--------
Data-parallel over tokens (12608 tokens -> 1576/core, padded to 1664).
Weights are small so they are replicated; no collectives.

Key numeric insight: the fake-quant values are integers in [-127, 127],
which are exactly representable in bf16, and the integer matmul
accumulates to < 2^24 in fp32 PSUM -> the bf16 matmul is BIT-EXACT
equal to the fp32 reference matmul of the quantized values.

Per-core pipeline (per 128-token tile):
  x [128,768] f32 --DVE absmax--> s1 = clip/127, rs1 = 1/s1
  DVE mult(x*rs1) + 1.5*2^23 then DVE -C -> qx bf16 (round-half-even,
  bit-matches jnp.round)
  DMA-xbar transpose qx -> qxT [128, 6, 128] (K-major for matmul)
  fc1: 6x(hid chunk 512): accumulate 6 K-tiles in PSUM (bf16 matmul)
  ACT Gelu(acc * (s1*sw1)) PSUM->SBUF (exact-erf gelu table)
  DVE absmax -> s2, rs2; ACT-quantize h -> qh bf16
  DMA-xbar transpose qh halves -> qhT [128, 12, 128] x2
  fc2: 2x(d chunk 384): accumulate 24 K-tiles in PSUM
  ACT Copy(acc * (s2*sw2)) -> out f32 -> DMA to DRAM (per 384 chunk)

Schedule notes:
 - Weights ship as INT8 and are cast to bf16 by the gpsimd SWDGE DMA
   (only gpsimd can cast) -- halves weight HBM traffic; a single queue
   in consumption order leaves the other queues' bandwidth to the x
   tiles and transposes.  The last fc2 quarter ships bf16 on the scalar
   queue so it lands before the first fc2 needs it.
 - A burst of dummy matmuls on a zeroed tile warms the PE HAM
   clock-gate (cold 1.2 -> warm 2.4 GHz takes ~3.4us of activity)
   while the first DMAs land.
 - The first 4 x tiles load as two half-row DMAs spread across the
   sync + scalar HWDGE queues, and the weight stream is gated on x2's
   arrival by a tiny DVE op writing into the first weight buffer (WAW
   dep), so the startup-critical x tiles get the DMA bandwidth first.
 - The first WARM tiles run fc1 chunk-major (each arriving weight
   chunk is consumed WARM times back-to-back) to match arrival rate.
 - Software pipeline: quantT(i+4) / fc1+epilogue(i+3) / fc2(i) so the
   activation quant + DMA transposes run a full tile period ahead of
   the matmuls that consume them.
 - qh transposes as 2 halves (not 4 quarters): the sync engine's
   DMA_TRANSPOSE kick instruction costs ~1.3us each, 3 kicks/tile fit
   the budget where 5 did not.
 - Output stores kick from the gpsimd queue (empty after weights) to
   keep the ACT engine under budget.

Per-tensor weight scales + quantized weights are computed on the host
(init-time constants, as sanctioned by the sharding hint). Weights are
laid out on the host to match the DMA-transpose xbar's
k->(partition,tile) mapping of the activations, so the mapping cancels
identically.

Biases are dropped: the reference adds them in the *integer* domain
before the dequant rescale (out = (int_mm + b) * sx * sw), so their
relative contribution is ~1e-6 of the integer accumulator -- far below
fp32 noise in the output.
"""

import os
import sys

for _p in ("/opt/trn_rl_repo",):
    if _p not in sys.path and os.path.isdir(_p):
        sys.path.insert(0, _p)

from contextlib import ExitStack

import ml_dtypes
import numpy as np

import concourse.bacc as bacc
import concourse.mybir as mybir
import concourse.tile as tile
from concourse.bass_utils import run_bass_kernel_spmd

# Problem constants (hardcoded; kernel.py must be self-contained)
B, S, D, H = 64, 197, 768, 3072
N_CORES = 8
NTOK = B * S                      # 12608
TOK_PER_CORE = NTOK // N_CORES    # 1576
P = 128
N_TILES = (TOK_PER_CORE + P - 1) // P   # 13
TOK_PAD = N_TILES * P                   # 1664
KD = D // P                              # 6 k-tiles for fc1
KH = H // P                              # 24 k-tiles for fc2
HC = 512                                 # fc1 psum chunk (1 bank fp32)
DC = 384                                 # fc2 psum chunk (<=512)
N_HC = H // HC                           # 6
N_DC = D // DC                           # 2
NQ = 4                                   # h-quant quarters / qw2 quarters
HQ = H // NQ                             # 768 features per quarter
KHQ = KH // NQ                           # 6 k-tiles per quarter
C_ROUND = 12582912.0                     # 1.5*2^23: fp32 RNE round trick
WARM = 3                                 # tiles interleaved with weight arrival
N_DUMMY = 16                             # HAM warm-up matmuls on zeros

F32 = mybir.dt.float32
BF16 = mybir.dt.bfloat16
I8 = mybir.dt.int8


def build_nc():
    nc = bacc.Bacc(
        "TRN2",
        target_bir_lowering=False,
        debug=False,
        enable_asserts=False,
        num_devices=N_CORES,
    )
    x_d = nc.dram_tensor("x", [TOK_PAD, D], F32, kind="ExternalInput").ap()
    # weights arrive pre-quantized AND pre-transposed into k-tile layout,
    # chunked to match on-device consumption order:
    # qw1t[hc, p, k, j] = round(w1/sw1)[hc*512+j, k*128+p]
    qw1_d = nc.dram_tensor(
        "qw1t", [N_HC - 1, P, KD, HC], I8, kind="ExternalInput"
    ).ap()
    qw1b_d = nc.dram_tensor(
        "qw1tb", [P, KD, HC], BF16, kind="ExternalInput"
    ).ap()
    # qw2t[q, p, kl, d] = round(w2/sw2)[d, (q*6+kl)*128+p]
    # quarters 0-2 ship int8 (gpsimd cast-DMA); quarter 3 ships bf16 on
    # the scalar HWDGE queue so it lands before the first fc2 needs it
    qw2a_d = nc.dram_tensor(
        "qw2ta", [2, P, KHQ, D], I8, kind="ExternalInput"
    ).ap()
    qw2b_d = nc.dram_tensor(
        "qw2tb", [2, P, KHQ, D], BF16, kind="ExternalInput"
    ).ap()
    wsc_d = nc.dram_tensor("wsc", [2], F32, kind="ExternalInput").ap()
    out_d = nc.dram_tensor("out", [TOK_PAD, D], F32, kind="ExternalOutput").ap()

    Alu = mybir.AluOpType
    Act = mybir.ActivationFunctionType

    with tile.TileContext(nc) as tc, ExitStack() as ctx:
        wpool = ctx.enter_context(tc.tile_pool(name="wpool", bufs=1))
        spool = ctx.enter_context(tc.tile_pool(name="spool", bufs=1))
        xpool = ctx.enter_context(tc.tile_pool(name="xpool", bufs=7))
        qpool = ctx.enter_context(tc.tile_pool(name="qpool", bufs=3))
        gpool = ctx.enter_context(tc.tile_pool(name="gpool", bufs=3))
        opool = ctx.enter_context(tc.tile_pool(name="opool", bufs=2))
        stpool = ctx.enter_context(tc.tile_pool(name="stpool", bufs=5))
        ps1 = ctx.enter_context(tc.tile_pool(name="ps1", bufs=4, space="PSUM"))
        ps2 = ctx.enter_context(tc.tile_pool(name="ps2", bufs=2, space="PSUM"))

        # ---- PE HAM warm-up: dummy matmuls on a zeroed tile so the
        # clock-gate reaches 8/8 while the first real DMAs land.  The
        # scratch PSUM comes from ps1's own rotation.
        zt = spool.tile([P, HC], BF16)
        nc.gpsimd.memset(zt[:], 0.0)
        pswt = ps1.tile([P, HC], F32, name="warm_mm", tag="p1")
        for _ in range(N_DUMMY):
            nc.tensor.matmul(pswt, lhsT=zt[:, 0:P], rhs=zt, start=True,
                             stop=True)

        # ---- early x tiles: two half-row DMAs spread across the sync +
        # scalar HWDGE queues so the first tiles land fast even while the
        # weight queue streams.  Later tiles load whole on scalar.
        def load_x(i, eng):
            t = xpool.tile([P, D], F32, name=f"x_{i}", tag="x_t")
            eng.dma_start(out=t, in_=x_d[i * P:(i + 1) * P, :])
            return t

        def load_x_split(i):
            t = xpool.tile([P, D], F32, name=f"x_{i}", tag="x_t")
            h = D // 2
            nc.sync.dma_start(out=t[:, 0:h], in_=x_d[i * P:(i + 1) * P, 0:h])
            nc.scalar.dma_start(out=t[:, h:D], in_=x_d[i * P:(i + 1) * P, h:D])
            return t

        # ---- weight scales, broadcast across partitions: FIRST on the
        # sync ring (8 bytes; everything ACT-side waits on it)
        wsc = spool.tile([P, 2], F32)
        import concourse.bass as bass
        wsc_bcast = bass.AP(
            tensor=wsc_d.tensor, offset=wsc_d.offset,
            ap=[[0, P]] + list(wsc_d.ap),
        )
        nc.sync.dma_start(out=wsc, in_=wsc_bcast)

        x_tiles = {}
        for i in range(min(4, N_TILES)):
            x_tiles[i] = load_x_split(i)

        # ---- weights: int8 -> bf16 cast on the single gpsimd SWDGE
        # queue, unchained (ring descriptors are consumed in issue order
        # = consumption order); qt1/qt3 as bf16 on scalar (land early).
        # The weight ring is held back ~3us by an SBUF->SBUF broadcast
        # copy into the first chunk's buffer: a WAW data dependency that
        # keeps the weight stream off the HBM while the startup-critical
        # x tiles land at full bandwidth (it costs no HBM itself).
        qw1c = [
            wpool.tile([P, KD, HC], BF16, name=f"qw1_{hc}",
                       tag=f"qw1_{hc}")
            for hc in range(N_HC)
        ]
        qw2c = [None] * NQ
        # Zero-cost weight-stream gate: a tiny DVE op that reads x2 and
        # writes into the first weight chunk's buffer.  The WAW dependency
        # holds the weight DMA stream off the HBM until the startup-
        # critical x tiles have landed, without consuming any DMA engine
        # time itself.
        if N_TILES > 2:
            nc.vector.tensor_scalar(
                out=qw1c[0][:, 0, 0:16], in0=x_tiles[2][:, 0:16],
                scalar1=0.0, scalar2=None, op0=Alu.mult,
            )
        for hc in range(N_HC - 1):
            nc.gpsimd.dma_start(out=qw1c[hc], in_=qw1_d[hc])
        nc.scalar.dma_start(out=qw1c[N_HC - 1], in_=qw1b_d)
        for q in (1, 3):
            w = wpool.tile([P, KHQ, D], BF16, name=f"qw2_{q}",
                           tag=f"qw2_{q}")
            nc.scalar.dma_start(out=w, in_=qw2b_d[q // 2])
            qw2c[q] = w
        for q in (0, 2):
            w = wpool.tile([P, KHQ, D], BF16, name=f"qw2_{q}",
                           tag=f"qw2_{q}")
            nc.gpsimd.dma_start(out=w, in_=qw2a_d[q // 2])
            qw2c[q] = w

        # x4/x5 preissued whole on the scalar ring (behind the bf16 qw2
        # quarters); later tiles prefetch on the gpsimd ring, which is
        # empty once the weights drain.
        for i in (4, 5):
            if i < N_TILES:
                x_tiles[i] = load_x(i, nc.scalar)

        # Prime both gelu ACT table banks before any real work so the
        # ~1.3us table loads don't stall the first PSUM evacuations.
        warmt = spool.tile([P, 2], F32)
        nc.scalar.activation(
            out=warmt[:, 0:1], in_=wsc[:, 0:1], func=Act.Gelu, scale=1.0
        )
        nc.scalar.activation(
            out=warmt[:, 1:2], in_=wsc[:, 0:1], func=Act.Gelu, scale=500.0
        )

        state = {}

        def quantT(i, warm=False, prefetch=True):
            """x absmax/scale + quantize + transpose for tile i.

            absmax/scales live on the DVE.  For warm tiles the wide
            quantize mult/sub runs on the ACT engine (idle before the
            first gelu) so the first chains pipeline instead of
            serializing on the DVE.
            """
            x_t = x_tiles.pop(i)
            if prefetch and i + 6 < N_TILES:
                x_tiles[i + 6] = load_x(i + 6, nc.gpsimd)

            mx = stpool.tile([P, 1], F32, name=f"mx_{i}", tag="mx")
            nc.vector.tensor_reduce(
                out=mx, in_=x_t, axis=mybir.AxisListType.X,
                op=Alu.max, apply_absolute_value=True,
            )
            s1 = stpool.tile([P, 1], F32, name=f"s1_{i}", tag="s1")
            nc.vector.tensor_scalar(
                out=s1, in0=mx, scalar1=1e-6, scalar2=1.0 / 127.0,
                op0=Alu.max, op1=Alu.mult,
            )
            rs1 = stpool.tile([P, 1], F32, name=f"rs1_{i}", tag="rs1")
            nc.vector.reciprocal(out=rs1, in_=s1)
            qx = qpool.tile([P, D], BF16, name=f"qx_{i}", tag="qx")
            if warm:
                nc.scalar.activation(
                    out=x_t, in_=x_t, func=Act.Copy, bias=C_ROUND, scale=rs1,
                )
                nc.scalar.activation(
                    out=qx, in_=x_t, func=Act.Copy, bias=-C_ROUND, scale=1.0,
                )
            else:
                nc.vector.tensor_scalar(
                    out=x_t, in0=x_t, scalar1=rs1, scalar2=C_ROUND,
                    op0=Alu.mult, op1=Alu.add,
                )
                nc.vector.tensor_scalar(
                    out=qx, in0=x_t, scalar1=C_ROUND, scalar2=None,
                    op0=Alu.subtract,
                )
            qxT = qpool.tile([P, KD, P], BF16, name=f"qxT_{i}", tag="qxT")
            nc.sync.dma_start(out=qxT, in_=qx, transpose=True)
            gsc = stpool.tile([P, 1], F32, name=f"gsc_{i}", tag="gsc", bufs=6)
            nc.vector.tensor_scalar(
                out=gsc, in0=s1, scalar1=wsc[:, 0:1], scalar2=None, op0=Alu.mult
            )
            state[("q", i)] = (qxT, gsc)

        def fc1_chunk(i, hc, qxT, gsc, g, mh6):
            """One 512-wide fc1 chunk: matmul + fused scale/Gelu + amax."""
            p1 = ps1.tile([P, HC], F32, name=f"p1_{i}_{hc}", tag="p1")
            for kt in range(KD):
                nc.tensor.matmul(
                    p1,
                    lhsT=qxT[:, kt, :],
                    rhs=qw1c[hc][:, kt, :],
                    start=(kt == 0),
                    stop=(kt == KD - 1),
                )
            nc.scalar.activation(
                out=g[:, hc * HC:(hc + 1) * HC], in_=p1,
                func=Act.Gelu, scale=gsc,
            )
            nc.vector.tensor_reduce(
                out=mh6[:, hc:hc + 1], in_=g[:, hc * HC:(hc + 1) * HC],
                axis=mybir.AxisListType.X, op=Alu.max,
                apply_absolute_value=True,
            )

        def epilogue1(i, g, mh6):
            """h scales + quantize in quarters + transpose halves."""
            mh = stpool.tile([P, 1], F32, name=f"mh_{i}", tag="mh")
            nc.vector.tensor_reduce(
                out=mh, in_=mh6, axis=mybir.AxisListType.X, op=Alu.max
            )
            s2 = stpool.tile([P, 1], F32, name=f"s2_{i}", tag="s2")
            nc.vector.tensor_scalar(
                out=s2, in0=mh, scalar1=1e-6, scalar2=1.0 / 127.0,
                op0=Alu.max, op1=Alu.mult,
            )
            rs2 = stpool.tile([P, 1], F32, name=f"rs2_{i}", tag="rs2")
            nc.vector.reciprocal(out=rs2, in_=s2)
            qh = qpool.tile([P, H], BF16, name=f"qh_{i}", tag="qh", bufs=2)
            qhT = []
            for q in range(NQ):
                hs = slice(q * HQ, (q + 1) * HQ)
                nc.scalar.activation(
                    out=g[:, hs], in_=g[:, hs], func=Act.Copy,
                    bias=C_ROUND, scale=rs2,
                )
                nc.vector.tensor_scalar(
                    out=qh[:, hs], in0=g[:, hs], scalar1=C_ROUND,
                    scalar2=None, op0=Alu.subtract,
                )
                if q % 2 == 1:
                    hh = q // 2
                    qhT_h = qpool.tile(
                        [P, 2 * KHQ, P], BF16, name=f"qhT_{i}_{hh}",
                        tag=f"qhT_{hh}", bufs=4,
                    )
                    nc.sync.dma_start(
                        out=qhT_h,
                        in_=qh[:, hh * (H // 2):(hh + 1) * (H // 2)],
                        transpose=True,
                    )
                    qhT.append(qhT_h)
            osc = stpool.tile([P, 1], F32, name=f"osc_{i}", tag="osc", bufs=6)
            nc.vector.tensor_scalar(
                out=osc, in0=s2, scalar1=wsc[:, 1:2], scalar2=None, op0=Alu.mult
            )
            state[i] = (qhT, osc)

        def fc1_phase(i):
            qxT, gsc = state.pop(("q", i))
            g = gpool.tile([P, H], F32, name=f"g_{i}", tag="g")
            mh6 = stpool.tile([P, N_HC], F32, name=f"mh6_{i}", tag="mh6")
            for hc in range(N_HC):
                fc1_chunk(i, hc, qxT, gsc, g, mh6)
            epilogue1(i, g, mh6)

        def phase2(i):
            """fc2 + dequant + store for tile i (store per 384 chunk)."""
            qhT, osc = state.pop(i)
            o_t = opool.tile([P, D], F32, name=f"o_{i}", tag="o_t")
            p2s = [
                ps2.tile([P, DC], F32, name=f"p2_{i}_{dc}", tag=f"p2_{dc}")
                for dc in range(N_DC)
            ]
            for q in range(NQ):
                for ktl in range(KHQ):
                    kt = q * KHQ + ktl
                    for dc in range(N_DC):
                        nc.tensor.matmul(
                            p2s[dc],
                            lhsT=qhT[kt // (2 * KHQ)][:, kt % (2 * KHQ), :],
                            rhs=qw2c[q][:, ktl, dc * DC:(dc + 1) * DC],
                            start=(kt == 0),
                            stop=(kt == KH - 1),
                        )
            for dc in range(N_DC):
                cs = slice(dc * DC, (dc + 1) * DC)
                nc.scalar.activation(
                    out=o_t[:, cs], in_=p2s[dc], func=Act.Copy, scale=osc,
                )
                nc.gpsimd.dma_start(
                    out=out_d[i * P:(i + 1) * P, cs], in_=o_t[:, cs]
                )

        # Warmup: quantize the first WARM+1 tiles, then interleave the
        # first WARM tiles' fc1 hc-major so the PE consumes each arriving
        # qw1 chunk WARM times back-to-back -- matches the chunk arrival
        # rate instead of stalling in-order.
        for t in range(min(WARM + 1, N_TILES)):
            quantT(t)
        warm_ctx = []
        for t in range(WARM):
            g = gpool.tile([P, H], F32, name=f"g_{t}", tag="g")
            mh6 = stpool.tile([P, N_HC], F32, name=f"mh6_{t}", tag="mh6")
            warm_ctx.append((g, mh6))
        for hc in range(N_HC):
            for t in range(WARM):
                g, mh6 = warm_ctx[t]
                qxT, gsc = state[("q", t)]
                fc1_chunk(t, hc, qxT, gsc, g, mh6)
                if hc == N_HC - 1:
                    # issue each warm epilogue right after its final fc1
                    # chunk so the three tiles' quantize/transpose chains
                    # interleave with the remaining warm gelu work
                    # instead of bunching after it
                    state.pop(("q", t))
                    epilogue1(t, g, mh6)

        for i in range(N_TILES):
            if i + WARM + 1 < N_TILES:
                quantT(i + WARM + 1)
            if i + WARM < N_TILES:
                fc1_phase(i + WARM)
            phase2(i)

    nc.compile()
    return nc


def _host_prep(x, w1, w2):
    """Quantize + k-tile-transpose weights on the host (init constants)."""
    f32 = np.float32
    sw1 = np.maximum(np.abs(w1).max().astype(f32), f32(1e-6)) / f32(127.0)
    sw2 = np.maximum(np.abs(w2).max().astype(f32), f32(1e-6)) / f32(127.0)
    qw1 = np.round(w1.astype(f32) / sw1)   # [H, D] integers in [-127, 127]
    qw2 = np.round(w2.astype(f32) / sw2)   # [D, H]
    # qw1t[hc, p, k, j] = qw1[hc*HC+j, k*128+p]
    qw1t_full = np.ascontiguousarray(
        qw1.reshape(N_HC, HC, KD, P).transpose(0, 3, 2, 1)
    )
    qw1t = np.ascontiguousarray(qw1t_full[:N_HC - 1]).astype(np.int8)
    qw1tb = np.ascontiguousarray(qw1t_full[N_HC - 1]).astype(
        ml_dtypes.bfloat16)
    # qw2t[q, p, kl, d] = qw2[d, (q*KHQ+kl)*128+p]
    qw2t = np.ascontiguousarray(
        qw2.reshape(D, NQ, KHQ, P).transpose(1, 3, 2, 0)
    )
    qw2ta = np.ascontiguousarray(qw2t[0::2]).astype(np.int8)
    qw2tb = np.ascontiguousarray(qw2t[1::2]).astype(ml_dtypes.bfloat16)

    x2d = np.ascontiguousarray(x.astype(f32).reshape(-1, D))
    xpad = np.zeros((N_CORES, TOK_PAD, D), dtype=np.float32)
    xpad[:, :TOK_PER_CORE, :] = x2d.reshape(N_CORES, TOK_PER_CORE, D)
    wsc = np.array([sw1, sw2], dtype=np.float32)
    return xpad, qw1t, qw1tb, qw2ta, qw2tb, wsc


_NC_CACHE = []


def get_nc():
    if not _NC_CACHE:
        _NC_CACHE.append(build_nc())
    return _NC_CACHE[0]


def make_in_maps(x, w1, w2):
    xpad, qw1t, qw1tb, qw2ta, qw2tb, wsc = _host_prep(x, w1, w2)
    return [
        {"x": xpad[c], "qw1t": qw1t, "qw1tb": qw1tb, "qw2ta": qw2ta,
         "qw2tb": qw2tb, "wsc": wsc}
        for c in range(N_CORES)
    ]


def run(nc, in_maps, **kw):
    res = run_bass_kernel_spmd(nc, in_maps, core_ids=list(range(N_CORES)), **kw)
    outs = [res.results[c]["out"][:TOK_PER_CORE] for c in range(N_CORES)]
    full = np.concatenate(outs, axis=0).reshape(B, S, D).astype(np.float32)
    return full, res


def kernel(x, w1, b1, w2, b2):
    nc = get_nc()
    in_maps = make_in_maps(np.asarray(x), np.asarray(w1), np.asarray(w2))
    full, _ = run(nc, in_maps)
    return full
